# revision 1
# baseline (speedup 1.0000x reference)
"""Chamfer distance kernel for Trainium2 (Bass/Tile), SPMD over 8 NeuronCores.

Math (per batch b):
  dist[v,l] = ||x_v||^2 - 2 x_v.y_l + ||y_l||^2,  x=[1024,512], y=[512,512]
  out[b] = mean_v min_l dist + mean_l min_v dist

Strategy (fp8 DoubleRow + exp/log-sum-exp, v8):
  - Data-parallel over batch: 64 batches -> 8 cores x 8 batches.
  - Host packs xs = fp8_e4m3(-2x), ys = fp8_e4m3(y) in k-chunk layout
    [P, KC, N]. ||y_q||^2 rides 2 fp8 contraction rows (hi/lo residual,
    error < +-0.5) in a K=128 zero-padded aug matmul per v-chunk, so
    PSUM holds pm = q + b_l. ||x_q||^2 enters exactly via the ACT bias.
  - PE per v-chunk: 2 fp8 DoubleRow matmuls (K=512, ~2 rows/cycle steady
    state) + 1 padded aug. Nothing else runs on the PE.
  - ACT per v-chunk: w = exp(-beta*pm + beta*(shift - a_v)) =
    exp(beta*(shift - dist)), with accum_out giving
    acc1[p,m] = sum_l w  == exp-encoded soft-min_l dist (D1, one pass).
  - DVE: D2 = running elementwise MAX of w (exp is monotone decreasing
    in dist, so max w <-> min dist, exact) into a [P,2,NL] fp32
    accumulator pair, merged once per batch.
  - Device ships acc1 [P,MC] and the merged max tile [P,NL] per batch;
    the host finishes with ln in fp64:
      d1_v = shift - ln(acc1)/beta        (soft-min, bias ~ -0.1)
      d2_l = shift - ln(max_p w)/beta     (exact)
    out = mean(d1) + mean(d2).  (ACT's Ln table is ~43% off on HW, and
    these are only O(B*(NV+NL)) host flops.)
"""

import numpy as np

N_CORES = 8
B = 8          # batches per core
D = 512        # feature dim
NV = 1024      # video clips
NL = 512       # language tokens
P = 128        # partitions
KC = D // P    # contraction chunks = 4
MC = NV // P   # v chunks = 8

BETA = 0.25    # LSE sharpness: softmin bias ~ -ln(1.5)/beta ~ -1.6e-1
SHIFT = 900.0  # exp arg = beta*(shift - dist); max arg ~ 52 << fp32's 88

_CACHE = {}


def _build_bass():
    import concourse.bass as bass
    import concourse.mybir as mybir
    import concourse.tile as tile
    from concourse import bacc

    f32 = mybir.dt.float32
    f8 = mybir.dt.float8e4
    ALU = mybir.AluOpType
    AFT = mybir.ActivationFunctionType
    DR = mybir.MatmulPerfMode.DoubleRow

    nc = bacc.Bacc(None)
    xs_h = nc.declare_dram_parameter("xs", [B, P, KC, NV], f8, isOutput=False)
    ys_h = nc.declare_dram_parameter("ys", [B, P, KC, NL], f8, isOutput=False)
    as_h = nc.declare_dram_parameter("as_", [B, P, NV], f8, isOutput=False)
    am_h = nc.declare_dram_parameter("am", [B, P, NL], f8, isOutput=False)
    ab_h = nc.declare_dram_parameter("ab", [B, P, MC], f32, isOutput=False)
    acc_h = nc.declare_dram_parameter("acc", [B, P, MC], f32, isOutput=True)
    mx_h = nc.declare_dram_parameter("mx", [B, P, NL], f32, isOutput=True)

    with tile.TileContext(nc) as tc:
        with (
            tc.tile_pool(name="io", bufs=3) as io,
            tc.tile_pool(name="work", bufs=3) as work,
            tc.tile_pool(name="ps", bufs=3, space="PSUM") as ps,
        ):
            for b in range(B):
                xs_t = io.tile([P, KC, NV], f8, tag="xs")
                ys_t = io.tile([P, KC, NL], f8, tag="ys")
                as_t = io.tile([P, NV], f8, tag="as")
                am_t = io.tile([P, NL], f8, tag="am")
                ab_t = io.tile([P, MC], f32, tag="ab")
                nc.sync.dma_start(out=xs_t[:, :2], in_=xs_h[b, :, :2])
                nc.sync.dma_start(out=xs_t[:, 2:], in_=xs_h[b, :, 2:])
                nc.sync.dma_start(out=ys_t, in_=ys_h[b])
                nc.sync.dma_start(out=as_t, in_=as_h[b])
                nc.sync.dma_start(out=am_t, in_=am_h[b])
                nc.sync.dma_start(out=ab_t, in_=ab_h[b])

                acc1 = work.tile([P, MC], f32, tag="acc1", bufs=2)
                rt2 = work.tile([P, 2, NL], f32, tag="rt2", bufs=2)

                for pr in range(MC // 2):
                    pm2 = ps.tile([P, 2, NL], f32, tag="pm", bufs=3)
                    for j in range(2):
                        m = 2 * pr + j
                        pm = pm2[:, j, :]
                        for kt2 in range(2):
                            nc.tensor.matmul(
                                out=pm,
                                lhsT=xs_t[:, 2 * kt2 : 2 * kt2 + 2, m * P : (m + 1) * P],
                                rhs=ys_t[:, 2 * kt2 : 2 * kt2 + 2, :],
                                start=(kt2 == 0),
                                stop=False,
                                perf_mode=DR,
                            )
                        # b_l aug (hi/lo fp8 rows, K=128 zero-padded).
                        nc.tensor.matmul(
                            out=pm,
                            lhsT=as_t[:, m * P : (m + 1) * P],
                            rhs=am_t,
                            start=False,
                            stop=True,
                        )
                    for j in range(2):
                        m = 2 * pr + j
                        if pr == 0:
                            wdst = rt2[:, j, :]
                        else:
                            wdst = work.tile([P, NL], f32, tag="w", bufs=3)
                        # w = exp(beta*(shift - dist)); acc1[:,m] = sum_l w.
                        nc.scalar.activation(
                            out=wdst,
                            in_=pm2[:, j, :],
                            func=AFT.Exp,
                            bias=ab_t[:, m : m + 1],
                            scale=-BETA,
                            accum_out=acc1[:, m : m + 1],
                        )
                        # D2: running max (exact min-dist tracking under exp).
                        if pr > 0:
                            nc.vector.tensor_tensor(
                                out=rt2[:, j, :], in0=wdst, in1=rt2[:, j, :],
                                op=ALU.max,
                            )

                # Merge accumulator pair and ship per-batch results.
                rtf = work.tile([P, NL], f32, tag="rtf", bufs=2)
                nc.vector.tensor_tensor(
                    out=rtf, in0=rt2[:, 0, :], in1=rt2[:, 1, :], op=ALU.max
                )
                nc.sync.dma_start(out=acc_h[b], in_=acc1)
                nc.sync.dma_start(out=mx_h[b], in_=rtf)

    nc.finalize()
    return nc


def _get_bass():
    if "nc" not in _CACHE:
        _CACHE["nc"] = _build_bass()
    return _CACHE["nc"]


def _run(in_maps, trace=False):
    from concourse.bass_utils import run_bass_kernel_spmd

    nc = _get_bass()
    return run_bass_kernel_spmd(nc, in_maps, list(range(N_CORES)), trace=trace)


def make_in_maps(video_feat, lang_feat):
    import ml_dtypes

    f8 = ml_dtypes.float8_e4m3
    video = np.asarray(video_feat, dtype=np.float32)
    lang = np.asarray(lang_feat, dtype=np.float32)
    assert video.shape == (N_CORES * B, NV, D), video.shape
    assert lang.shape == (N_CORES * B, NL, D), lang.shape
    NB = N_CORES * B

    xs8 = (-2.0 * video).astype(f8)                      # [64, NV, D]
    ys8 = lang.astype(f8)                                # [64, NL, D]
    xsf = xs8.astype(np.float32)
    ysf = ys8.astype(np.float32)
    a = np.einsum("bvd,bvd->bv", xsf, xsf) / 4.0         # ||x_q||^2  [64, NV]
    bn = np.einsum("bld,bld->bl", ysf, ysf)              # ||y_q||^2  [64, NL]

    b_hi = (bn / 64.0).astype(f8)
    b_lo = (bn - 64.0 * b_hi.astype(np.float32)).astype(f8)

    # aug stationary [64, P, NV]: rows (64s, 1s), rest zero.
    as_dev = np.zeros((NB, P, NV), f8)
    as_dev[:, 0, :] = np.float32(64.0)
    as_dev[:, 1, :] = np.float32(1.0)
    # aug moving [64, P, NL]: rows (b_hi, b_lo), rest zero.
    am_dev = np.zeros((NB, P, NL), f8)
    am_dev[:, 0, :] = b_hi
    am_dev[:, 1, :] = b_lo

    # ACT bias: beta*(shift - a_v), laid out [P, MC].
    ab_dev = np.ascontiguousarray(
        (BETA * (SHIFT - a)).reshape(NB, MC, P).transpose(0, 2, 1)
    ).astype(np.float32)

    xs_dev = np.ascontiguousarray(
        xs8.reshape(NB, NV, KC, P).transpose(0, 3, 2, 1)
    )  # [64, P, KC, NV]
    ys_dev = np.ascontiguousarray(
        ys8.reshape(NB, NL, KC, P).transpose(0, 3, 2, 1)
    )  # [64, P, KC, NL]

    in_maps = []
    for c in range(N_CORES):
        sl = slice(c * B, (c + 1) * B)
        in_maps.append(
            {
                "xs": xs_dev[sl],
                "ys": ys_dev[sl],
                "as_": as_dev[sl],
                "am": am_dev[sl],
                "ab": ab_dev[sl],
            }
        )
    return in_maps


def finish(res):
    """Host finish: d1 = shift - ln(acc1)/beta (soft-min over l),
    d2 = shift - ln(max_p w)/beta (exact min over v), means summed."""
    outs = []
    for c in range(N_CORES):
        acc = res.results[c]["acc"].astype(np.float64)   # [B, P, MC]
        mx = res.results[c]["mx"].astype(np.float64)     # [B, P, NL]
        d1sum = np.log(acc).sum(axis=(1, 2))             # [B]
        d2sum = np.log(mx.max(axis=1)).sum(axis=1)       # [B]
        out = (SHIFT - d1sum / (BETA * NV)) + (SHIFT - d2sum / (BETA * NL))
        outs.append(out.astype(np.float32))
    return np.concatenate(outs)


def kernel(video_feat, lang_feat):
    in_maps = make_in_maps(video_feat, lang_feat)
    res = _run(in_maps, trace=False)
    return finish(res).astype(np.float32)

